# revision 4
# baseline (speedup 1.0000x reference)
"""Trainium2 Bass kernel for 3x3 conv (stride 1, pad 1) + bias.

Problem: x (32,128,56,56) f32, weights (256,128,3,3) f32, bias (256,) f32
         -> out (32,256,56,56) f32.

Strategy: data-parallel over batch (4 images per core, 8 cores).
Per core: implicit GEMM. C_in=128 lives on the SBUF partition axis (the
matmul contraction dim). Each image is stored width+height zero-padded
(58x58 grid) in a flat per-image slot so that, for every 3x3 tap (kh,kw),
the conv becomes ONE shifted matmul over 8 output rows accumulated in
PSUM across the 9 taps. C_out=256 is split into two 128-partition halves
(the matmul M dim). Bias is added during PSUM->SBUF eviction (Act).

Taps that would only read zero padding are range-restricted: kh=0/kh=2
skip the first/last output row of edge chunks, kw=0/kw=2 skip the
first/last output column everywhere (saves ~2.2us of PE streaming).

Head: the PE clock is HAM-gated (4/8 until ~3.4us of sustained
activity) and input DMAs have ~3.5us trigger-to-ready latency, so a
junk-matmul bridge (gated only on tiny gpsimd memsets) keeps the PE
busy from ~7us until the first real operands land (~10.5us).

Tail: evictions stage into a per-image-half SBUF tile with one DMA per
half; the final image-half keeps per-chunk DMAs and tapers its last
chunks (8..8,4,2,2 rows) so the end-of-kernel chain hangs off a tiny
2-row group evicted by the otherwise-idle Vector engine.
"""

import os
from contextlib import ExitStack

import ml_dtypes
import numpy as np

import concourse.bacc as bacc
import concourse.bass as bass
import concourse.mybir as mybir
import concourse.tile as tile
import concourse.bass_utils as bass_utils

N_CORES = 8
B, CIN, H, W = 32, 128, 56, 56
COUT = 256
BPC = B // N_CORES          # images per core
PW, PH = W + 1, H + 2       # grid 58 rows x 57 cols: one shared pad col
GRID = PW * PH              # 3306  (col 0 of each row is the zero pad;
                            #  col 57 === next row's col 0)
SLOT = GRID + 2             # +2 zero guard for the last row's col-57 read
RPC = 8                     # output rows per PSUM chunk
NCHUNK = H // RPC           # 7
NFREE = RPC * W             # 448
KK = 9

P1 = 576                    # first x piece: covers grid rows 0..9

DT = mybir.dt.bfloat16
NPDT = ml_dtypes.bfloat16

_CACHE: dict = {}

# tap order: kh=1 first (tap 4 is full-range -> start), then kh=0, kh=2
TAP_ORDER = (4, 3, 5, 1, 0, 2, 7, 6, 8)


def _build():
    """Build the per-core Bass program (same program on all 8 cores)."""
    nc = bacc.Bacc("TRN2", target_bir_lowering=False, debug=False,
                   num_devices=N_CORES)
    f32 = mybir.dt.float32
    xp = nc.dram_tensor("xp", [BPC, CIN, SLOT], DT, kind="ExternalInput").ap()
    # wa: kh=1 taps (3,4,5); wb: taps (0,1,2,6,7,8) in that slot order
    wa = nc.dram_tensor("wa", [CIN, 3 * COUT], DT, kind="ExternalInput").ap()
    wb = nc.dram_tensor("wb", [CIN, 6 * COUT], DT, kind="ExternalInput").ap()
    b2 = nc.dram_tensor("b2", [128, 2], f32, kind="ExternalInput").ap()
    out = nc.dram_tensor("out", [BPC, COUT, H, W], f32,
                         kind="ExternalOutput").ap()

    with tile.TileContext(nc) as tc, ExitStack() as ctx:
        const_pool = ctx.enter_context(tc.tile_pool(name="const", bufs=1))
        xpool = ctx.enter_context(tc.tile_pool(name="xp_pool", bufs=1))
        epool = ctx.enter_context(tc.tile_pool(name="epool", bufs=2))
        tpool = ctx.enter_context(tc.tile_pool(name="tpool", bufs=3))
        psum = ctx.enter_context(
            tc.tile_pool(name="psum", bufs=7, space="PSUM"))
        wupool = ctx.enter_context(
            tc.tile_pool(name="wupool", bufs=1, space="PSUM"))

        wbufA = const_pool.tile([CIN, 3 * COUT], DT)
        wbufB = const_pool.tile([CIN, 6 * COUT], DT)
        xbuf = xpool.tile([CIN, BPC * SLOT], DT)
        bbuf = const_pool.tile([128, 2], f32)

        # HAM warmup bridge: junk matmuls gated only on gpsimd memsets.
        # Small memset first so the PE activity window opens ASAP; the
        # bridge spans until the first real operands are DMA-ready.
        wrmA = const_pool.tile([128, 128], DT)
        wrmB = const_pool.tile([128, 512], DT)
        nc.gpsimd.memset(wrmA[:], 0)
        nc.gpsimd.memset(wrmB[:], 0)
        wps = wupool.tile([128, 512], f32)
        for _ in range(3):
            nc.tensor.matmul(wps[:, :128], wrmA[:], wrmA[:],
                             start=True, stop=True)
        for _ in range(7):
            nc.tensor.matmul(wps[:], wrmA[:], wrmB[:],
                             start=True, stop=True)

        # DMA-in. scalar queue: image 0 in three pieces (small first).
        # sync queue: weights (kh=1 tile first), bias, then images 1-3.
        nc.scalar.dma_start(xbuf[:, :P1], xp[0][:, :P1])
        nc.scalar.dma_start(xbuf[:, P1:1654], xp[0][:, P1:1654])
        nc.sync.dma_start(wbufA[:], wa)
        nc.sync.dma_start(bbuf[:], b2)
        nc.sync.dma_start(wbufB[:], wb)
        nc.scalar.dma_start(xbuf[:, 1654:SLOT], xp[0][:, 1654:SLOT])
        hs = SLOT // 2
        for n in range(1, BPC):
            for lo, hi in ((0, hs), (hs, SLOT)):
                nc.sync.dma_start(
                    xbuf[:, n * SLOT + lo:n * SLOT + hi],
                    xp[n][:, lo:hi])

        def wsl(k, h):
            if k in (3, 4, 5):
                c0 = (k - 3) * COUT + h * 128
                return wbufA[:, c0:c0 + 128]
            slot = k if k < 3 else k - 3
            c0 = slot * COUT + h * 128
            return wbufB[:, c0:c0 + 128]

        pss = [psum.tile([128, NFREE], f32, name=f"ps{i}", tag=f"ps{i}",
                         bufs=1)
               for i in range(NCHUNK)]

        def group(n, h, ps, r0, r1):
            """Accumulate the 9 pad-restricted taps for out rows [r0,r1)."""
            R = r1 - r0
            ps3 = ps[:, :R * W].rearrange("p (r c) -> p r c", c=W)
            for i, k in enumerate(TAP_ORDER):
                kh, kw = divmod(k, 3)
                r_lo = max(r0, 1) if kh == 0 else r0
                r_hi = min(r1, H - 1) if kh == 2 else r1
                dest = ps3[:, r_lo - r0:r_hi - r0]
                s = n * SLOT + PW * (r_lo + kh) + kw
                rhs = xbuf[:, s:s + (r_hi - r_lo) * PW].rearrange(
                    "p (r c) -> p r c", c=PW)[:, :, :W]
                nc.tensor.matmul(dest, wsl(k, h), rhs,
                                 start=(i == 0), stop=(i == KK - 1))

        def evict_act(dst, ps, R, h):
            nc.scalar.activation(
                dst, ps[:, :R * W],
                mybir.ActivationFunctionType.Identity,
                bias=bbuf[:, h:h + 1])

        for n in range(BPC):
            for h in range(2):
                last_half = (n == BPC - 1 and h == 1)
                if not last_half:
                    evh = epool.tile([128, NCHUNK * NFREE], f32)
                    for c in range(NCHUNK):
                        group(n, h, pss[c], c * RPC, (c + 1) * RPC)
                        evict_act(evh[:, c * NFREE:(c + 1) * NFREE],
                                  pss[c], RPC, h)
                    od = out[n, h * 128:(h + 1) * 128].rearrange(
                        "c r w -> c (r w)")
                    nc.scalar.dma_start(od, evh[:])
                else:
                    # final image-half: per-chunk DMAs, tapered last chunks
                    # (rows 48-51, 52-53, 54-55) so the end-of-kernel chain
                    # hangs off a tiny group; last eviction on Vector (DVE)
                    # + last DMA on the idle sync queue.
                    pieces = [(c * RPC, (c + 1) * RPC, pss[c], "act", "sc")
                              for c in range(NCHUNK - 1)]
                    pieces += [(48, 52, pss[6], "act", "sc"),
                               (52, 54, pss[0], "act", "sc"),
                               (54, 56, pss[1], "dve", "sy")]
                    for r0, r1, ps, eng, q in pieces:
                        R = r1 - r0
                        group(n, h, ps, r0, r1)
                        ev = tpool.tile([128, R * W], f32)
                        if eng == "act":
                            evict_act(ev[:], ps, R, h)
                        else:
                            nc.vector.tensor_scalar_add(
                                ev[:], ps[:, :R * W], bbuf[:, h:h + 1])
                        od = out[n, h * 128:(h + 1) * 128,
                                 r0:r1].rearrange("c r w -> c (r w)")
                        if q == "sy":
                            nc.sync.dma_start(od, ev[:])
                        else:
                            nc.scalar.dma_start(od, ev[:])
    nc.compile()
    return nc


def _prep(x, weights, bias):
    """Host-side reshape/pad/cast into the device layouts."""
    xpad = np.zeros((B, CIN, SLOT), dtype=NPDT)
    grid = xpad[:, :, :GRID].reshape(B, CIN, PH, PW)
    # rows 1..56 hold the image; col 0 is the zero pad column (col 57 of a
    # row aliases the next row's col 0, so one pad column serves both edges)
    grid[:, :, 1:1 + H, 1:1 + W] = np.asarray(x).astype(NPDT)
    # weights (co, ci, kh, kw) -> (ci, kh*kw*co) flat, split A (kh=1) / B
    wt = np.ascontiguousarray(
        np.asarray(weights).transpose(1, 2, 3, 0)).reshape(
            CIN, KK * COUT).astype(NPDT)
    wtA = np.ascontiguousarray(wt[:, 3 * COUT:6 * COUT])
    wtB = np.ascontiguousarray(
        np.concatenate([wt[:, :3 * COUT], wt[:, 6 * COUT:]], axis=1))
    b2 = np.ascontiguousarray(
        np.asarray(bias).astype(np.float32).reshape(2, 128).T)
    return xpad, wtA, wtB, b2


def kernel(x, weights, bias):
    if "nc" not in _CACHE:
        _CACHE["nc"] = _build()
    nc = _CACHE["nc"]
    xpad, wtA, wtB, b2 = _prep(x, weights, bias)
    in_maps = [
        {"xp": xpad[i * BPC:(i + 1) * BPC], "wa": wtA, "wb": wtB, "b2": b2}
        for i in range(N_CORES)
    ]
    res = bass_utils.run_bass_kernel_spmd(
        nc, in_maps, core_ids=list(range(N_CORES)),
        trace=bool(int(os.environ.get("CONV_TRACE", "0"))),
    )
    if os.environ.get("CONV_TRACE"):
        _CACHE["last_result"] = res
    return np.concatenate([r["out"] for r in res.results], axis=0)


# revision 6
# speedup vs baseline: 1.0603x; 1.0603x over previous
"""Trainium2 Bass kernel for 3x3 conv (stride 1, pad 1) + bias.

Problem: x (32,128,56,56) f32, weights (256,128,3,3) f32, bias (256,) f32
         -> out (32,256,56,56) f32.

Strategy: data-parallel over batch (4 images per core, 8 cores).
Per core: implicit GEMM. C_in=128 lives on the SBUF partition axis (the
matmul contraction dim). Each image is stored width+height zero-padded
(58x58 grid) in a flat per-image slot so that, for every 3x3 tap (kh,kw),
the conv becomes ONE shifted matmul over 8 output rows accumulated in
PSUM across the 9 taps. C_out=256 is split into two 128-partition halves
(the matmul M dim). Bias is added during PSUM->SBUF eviction (Act).

Taps that would only read zero padding are range-restricted: kh=0/kh=2
skip the first/last output row of edge chunks, kw=0/kw=2 skip the
first/last output column everywhere (saves ~2.2us of PE streaming).

Head: the PE clock is HAM-gated (4/8 until ~3.4us of sustained
activity) and input DMAs have ~3.5us trigger-to-ready latency, so a
junk-matmul bridge (gated only on tiny gpsimd memsets) keeps the PE
busy from ~7us until the first real operands land (~10.5us).

Tail: evictions stage into a per-image-half SBUF tile with one DMA per
half; the final image-half keeps per-chunk DMAs and tapers its last
chunks (8..8,4,2,2 rows) so the end-of-kernel chain hangs off a tiny
2-row group evicted by the otherwise-idle Vector engine.
"""

import os
from contextlib import ExitStack

import ml_dtypes
import numpy as np

import concourse.bacc as bacc
import concourse.bass as bass
import concourse.mybir as mybir
import concourse.tile as tile
import concourse.bass_utils as bass_utils

N_CORES = 8
B, CIN, H, W = 32, 128, 56, 56
COUT = 256
BPC = B // N_CORES          # images per core
PW, PH = W + 1, H + 2       # grid 58 rows x 57 cols: one shared pad col
GRID = PW * PH              # 3306  (col 0 of each row is the zero pad;
                            #  col 57 === next row's col 0)
SLOT = GRID + 2             # +2 zero guard for the last row's col-57 read
RPC = 8                     # output rows per PSUM chunk
NCHUNK = H // RPC           # 7
NFREE = RPC * W             # 448
KK = 9

P1 = 576                    # first x piece: covers grid rows 0..9

DT = mybir.dt.bfloat16
NPDT = ml_dtypes.bfloat16

_CACHE: dict = {}

# tap order: kh=1 first (tap 4 is full-range -> start), then kh=0, kh=2
TAP_ORDER = (4, 3, 5, 1, 0, 2, 7, 6, 8)


def _build():
    """Build the per-core Bass program (same program on all 8 cores)."""
    nc = bacc.Bacc("TRN2", target_bir_lowering=False, debug=False,
                   num_devices=N_CORES)
    f32 = mybir.dt.float32
    xp = nc.dram_tensor("xp", [BPC, CIN, SLOT], DT, kind="ExternalInput").ap()
    # wa: kh=1 taps (3,4,5); wb: taps (0,1,2,6,7,8) in that slot order
    wa = nc.dram_tensor("wa", [CIN, 3 * COUT], DT, kind="ExternalInput").ap()
    wb = nc.dram_tensor("wb", [CIN, 6 * COUT], DT, kind="ExternalInput").ap()
    b2 = nc.dram_tensor("b2", [128, 2], f32, kind="ExternalInput").ap()
    out = nc.dram_tensor("out", [BPC, COUT, H, W], f32,
                         kind="ExternalOutput").ap()

    with tile.TileContext(nc) as tc, ExitStack() as ctx:
        const_pool = ctx.enter_context(tc.tile_pool(name="const", bufs=1))
        xpool = ctx.enter_context(tc.tile_pool(name="xp_pool", bufs=1))
        epool = ctx.enter_context(tc.tile_pool(name="epool", bufs=2))
        tpool = ctx.enter_context(tc.tile_pool(name="tpool", bufs=5))
        psum = ctx.enter_context(
            tc.tile_pool(name="psum", bufs=7, space="PSUM"))
        wupool = ctx.enter_context(
            tc.tile_pool(name="wupool", bufs=1, space="PSUM"))

        wbufA = const_pool.tile([CIN, 3 * COUT], DT)
        wbufB = const_pool.tile([CIN, 6 * COUT], DT)
        xbuf = xpool.tile([CIN, BPC * SLOT], DT)
        bbuf = const_pool.tile([128, 2], f32)

        # HAM warmup bridge: junk matmuls gated only on gpsimd memsets.
        # Small memset first so the PE activity window opens ASAP; the
        # bridge spans until the first real operands are DMA-ready.
        wrmA = const_pool.tile([128, 128], DT)
        wrmB = const_pool.tile([128, 512], DT)
        nc.gpsimd.memset(wrmA[:], 0)
        nc.gpsimd.memset(wrmB[:], 0)
        wps = wupool.tile([128, 512], f32)
        for _ in range(3):
            nc.tensor.matmul(wps[:, :128], wrmA[:], wrmA[:],
                             start=True, stop=True)
        for _ in range(8):
            nc.tensor.matmul(wps[:], wrmA[:], wrmB[:],
                             start=True, stop=True)

        # DMA-in. The sync queue's HW queue starts moving data ~0.8us
        # before the scalar one, so everything the first chunks need rides
        # sync in consumption order; scalar only carries the second weight
        # tile (needed a few matmuls in) and later the output DMAs.
        nc.sync.dma_start(wbufA[:], wa)
        nc.sync.dma_start(xbuf[:, :P1], xp[0][:, :P1])
        nc.sync.dma_start(bbuf[:], b2)
        nc.scalar.dma_start(wbufB[:], wb)
        nc.sync.dma_start(xbuf[:, P1:1654], xp[0][:, P1:1654])
        nc.sync.dma_start(xbuf[:, 1654:2481], xp[0][:, 1654:2481])
        nc.sync.dma_start(xbuf[:, 2481:SLOT], xp[0][:, 2481:SLOT])
        hs = SLOT // 2
        for n in range(1, BPC):
            for lo, hi in ((0, hs), (hs, SLOT)):
                nc.sync.dma_start(
                    xbuf[:, n * SLOT + lo:n * SLOT + hi],
                    xp[n][:, lo:hi])

        def wsl(k, h):
            if k in (3, 4, 5):
                c0 = (k - 3) * COUT + h * 128
                return wbufA[:, c0:c0 + 128]
            slot = k if k < 3 else k - 3
            c0 = slot * COUT + h * 128
            return wbufB[:, c0:c0 + 128]

        pss = [psum.tile([128, NFREE], f32, name=f"ps{i}", tag=f"ps{i}",
                         bufs=1)
               for i in range(NCHUNK)]

        def group(n, h, ps, r0, r1):
            """Accumulate the 9 pad-restricted taps for out rows [r0,r1)."""
            R = r1 - r0
            ps3 = ps[:, :R * W].rearrange("p (r c) -> p r c", c=W)
            for i, k in enumerate(TAP_ORDER):
                kh, kw = divmod(k, 3)
                r_lo = max(r0, 1) if kh == 0 else r0
                r_hi = min(r1, H - 1) if kh == 2 else r1
                dest = ps3[:, r_lo - r0:r_hi - r0]
                s = n * SLOT + PW * (r_lo + kh) + kw
                rhs = xbuf[:, s:s + (r_hi - r_lo) * PW].rearrange(
                    "p (r c) -> p r c", c=PW)[:, :, :W]
                nc.tensor.matmul(dest, wsl(k, h), rhs,
                                 start=(i == 0), stop=(i == KK - 1))

        def evict_act(dst, ps, R, h):
            nc.scalar.activation(
                dst, ps[:, :R * W],
                mybir.ActivationFunctionType.Identity,
                bias=bbuf[:, h:h + 1])

        for n in range(BPC):
            for h in range(2):
                last_half = (n == BPC - 1 and h == 1)
                if not last_half:
                    evh = epool.tile([128, NCHUNK * NFREE], f32)
                    for c in range(NCHUNK):
                        group(n, h, pss[c], c * RPC, (c + 1) * RPC)
                        evict_act(evh[:, c * NFREE:(c + 1) * NFREE],
                                  pss[c], RPC, h)
                    od = out[n, h * 128:(h + 1) * 128].rearrange(
                        "c r w -> c (r w)")
                    nc.scalar.dma_start(od, evh[:])
                else:
                    # final image-half: per-chunk DMAs, tapered last chunks
                    # (rows 48-51, 52-53, 54-55) so the end-of-kernel chain
                    # hangs off a tiny group; last eviction on Vector (DVE)
                    # + last DMA on the idle sync queue.
                    pieces = [(c * RPC, (c + 1) * RPC, pss[c], "act", "sc")
                              for c in range(NCHUNK - 1)]
                    pieces += [(48, 52, pss[6], "act", "sc"),
                               (52, 54, pss[0], "act", "sc"),
                               (54, 56, pss[1], "dve", "sy")]
                    for r0, r1, ps, eng, q in pieces:
                        R = r1 - r0
                        group(n, h, ps, r0, r1)
                        ev = tpool.tile([128, R * W], f32)
                        if eng == "act":
                            evict_act(ev[:], ps, R, h)
                        else:
                            nc.vector.tensor_scalar_add(
                                ev[:], ps[:, :R * W], bbuf[:, h:h + 1])
                        od = out[n, h * 128:(h + 1) * 128,
                                 r0:r1].rearrange("c r w -> c (r w)")
                        if q == "sy":
                            nc.sync.dma_start(od, ev[:])
                        else:
                            nc.scalar.dma_start(od, ev[:])
    nc.compile()
    return nc


def _prep(x, weights, bias):
    """Host-side reshape/pad/cast into the device layouts."""
    xpad = np.zeros((B, CIN, SLOT), dtype=NPDT)
    grid = xpad[:, :, :GRID].reshape(B, CIN, PH, PW)
    # rows 1..56 hold the image; col 0 is the zero pad column (col 57 of a
    # row aliases the next row's col 0, so one pad column serves both edges)
    grid[:, :, 1:1 + H, 1:1 + W] = np.asarray(x).astype(NPDT)
    # weights (co, ci, kh, kw) -> (ci, kh*kw*co) flat, split A (kh=1) / B
    wt = np.ascontiguousarray(
        np.asarray(weights).transpose(1, 2, 3, 0)).reshape(
            CIN, KK * COUT).astype(NPDT)
    wtA = np.ascontiguousarray(wt[:, 3 * COUT:6 * COUT])
    wtB = np.ascontiguousarray(
        np.concatenate([wt[:, :3 * COUT], wt[:, 6 * COUT:]], axis=1))
    b2 = np.ascontiguousarray(
        np.asarray(bias).astype(np.float32).reshape(2, 128).T)
    return xpad, wtA, wtB, b2


def kernel(x, weights, bias):
    if "nc" not in _CACHE:
        _CACHE["nc"] = _build()
    nc = _CACHE["nc"]
    xpad, wtA, wtB, b2 = _prep(x, weights, bias)
    in_maps = [
        {"xp": xpad[i * BPC:(i + 1) * BPC], "wa": wtA, "wb": wtB, "b2": b2}
        for i in range(N_CORES)
    ]
    res = bass_utils.run_bass_kernel_spmd(
        nc, in_maps, core_ids=list(range(N_CORES)),
        trace=bool(int(os.environ.get("CONV_TRACE", "0"))),
    )
    if os.environ.get("CONV_TRACE"):
        _CACHE["last_result"] = res
    return np.concatenate([r["out"] for r in res.results], axis=0)
